# revision 17
# baseline (speedup 1.0000x reference)
"""Trainium2 Bass kernel for the BDH recurrent block (B=8, T=256, d=256, n=1024).

Key reformulation: the scan input v_prev is the *embedding* at each step (the
output v_star is never fed back), so the only recurrences are

  x_t = l1norm(0.97 * x_{t-1} + relu(emb_t @ Dx.T))          (elementwise, n)
  rho_t = 0.97 * rho_{t-1} + ln(emb_t) (x) x_t               (rank-1, d*n)

Both have closed forms:
  x_t  = sum_s C[t,s] * U_s           with U = relu(emb @ Dx.T)  and
         C[t,s] = 0.97^{t-s} / prod_{r=s..t} b_r,  b_r = sum(U_r) + 0.97*[r>0]
         (b_0 = sum(U_0)), computed in log space via a cumulative sum.
  a*_t = rho_{t-1} x_t = sum_{s<t} 0.97^{t-1-s} (x_s . x_t) ln(emb_s)
       = ((X X^T) o D) @ ln(emb)     -- decay-masked attention.

So the whole T-step scan becomes a handful of dense matmuls, one sample per
NeuronCore (data-parallel over B=8 across 8 cores, weights replicated).
"""

import numpy as np

import concourse.bass as bass
import concourse.tile as tile
from concourse import bacc, mybir
from concourse.bass_utils import run_bass_kernel_spmd
from concourse.hw_specs import get_activation_tables

B, T, D, N = 8, 256, 256, 1024
P = 128  # partitions
LN_EPS = 1e-5
DECAY = 0.97
F32 = mybir.dt.float32
F32R = mybir.dt.float32r
AF = mybir.ActivationFunctionType
ALU = mybir.AluOpType

# fp32r runs the PE at 4x the fp32 rate (1 cycle/row at N>=256); inputs are
# fp32 in SBUF, bitcast at the matmul. Used only for the big matmuls.
USE_F32R = True


def _mm(nc, out, lhsT, rhs, start, stop, fast):
    nc.tensor.matmul(out, lhsT, rhs, start=start, stop=stop)


def _build_nc(use_f32r=USE_F32R):
    nc = bacc.Bacc()
    FDT = F32R if use_f32r else F32

    # packed inputs (few large DMAs, ordered by when the pipeline needs them)
    d_sc = nc.dram_tensor("sc", [T, 4], F32, kind="ExternalInput")  # c097|iotaP|iotaQ|0
    d_tid = nc.dram_tensor("tid", [P, 2 * P], F32, kind="ExternalInput")  # triu|ident
    d_em2 = nc.dram_tensor("em2", [T, 2 * D], FDT, kind="ExternalInput")  # emb|embT
    d_DxT = nc.dram_tensor("DxT", [D, N], FDT, kind="ExternalInput")
    d_mask2 = nc.dram_tensor("mask2", [T, 2 * T], F32, kind="ExternalInput")  # maskCT|DupT
    d_DyT = nc.dram_tensor("DyT", [D, N], FDT, kind="ExternalInput")
    d_ET = nc.dram_tensor("ET", [N, D], FDT, kind="ExternalInput")
    d_out = nc.dram_tensor("out", [T, D], F32, kind="ExternalOutput")

    # Preload the one ACT table set containing every function we use
    # (relu/ln/exp/copy) so the compiler never swaps tables mid-kernel
    # (each swap costs ~2.7us on the Scalar engine).
    act_sets = list(get_activation_tables(nc.m.arch))
    combined_set_id = act_sets.index("natural_log_exp_and_others")

    with tile.TileContext(nc) as tc:
        nc.scalar.add_instruction(mybir.InstLoadActFuncSet(
            name=nc.get_next_instruction_name(),
            act_func_set_id=combined_set_id, ins=[], outs=[]))
        with (
            tc.tile_pool(name="consts", bufs=1) as cp,
            tc.tile_pool(name="work", bufs=1) as wp,
            tc.tile_pool(name="ps512", bufs=1, space="PSUM") as ps512,
            tc.tile_pool(name="ps256", bufs=5, space="PSUM") as ps256,
            tc.tile_pool(name="ps_small", bufs=2, space="PSUM") as pss,
        ):
            # ---- load inputs (issue order == need order) --------------------
            def load2(dram, f, tag, dt_=F32):  # (2P, f) dram -> two [P, f] tiles
                ts = []
                for k in range(2):
                    t = cp.tile([P, f], dt_, tag=f"{tag}{k}", name=f"{tag}{k}")
                    nc.sync.dma_start(t[:], dram[k * P:(k + 1) * P, :])
                    ts.append(t)
                return ts

            sc_s = load2(d_sc, 4, "sc")
            c097_s = [t[:, 0:1] for t in sc_s]
            iotaP_s = [t[:, 1:2] for t in sc_s]
            iotaQ_s = [t[:, 2:3] for t in sc_s]
            tid_s = cp.tile([P, 2 * P], F32, tag="tid", name="tid")
            nc.sync.dma_start(tid_s[:], d_tid[:, :])
            triu_s = tid_s[:, 0:P]
            ident_s = tid_s[:, P:2 * P]
            em2_s = load2(d_em2, 2 * D, "em2", FDT)
            emb_s = [t[:, 0:D].bitcast(F32) for t in em2_s]
            embT_s = [t[:, D:2 * D] for t in em2_s]
            DxT_s = load2(d_DxT, N, "DxT", FDT)
            def load2g(dram, f, tag, dt_=F32):
                ts = []
                for k in range(2):
                    t = cp.tile([P, f], dt_, tag=f"{tag}{k}", name=f"{tag}{k}")
                    nc.gpsimd.dma_start(t[:], dram[k * P:(k + 1) * P, :])
                    ts.append(t)
                return ts

            mask2_s = load2(d_mask2, 2 * T, "mask2")
            maskCT_s = [t[:, 0:T] for t in mask2_s]
            DupT_s = [t[:, T:2 * T] for t in mask2_s]
            DyT_s = load2(d_DyT, N, "DyT", FDT)
            et_big = cp.tile([P, 8, D], FDT, tag="et_big", name="et_big")
            nc.sync.dma_start(
                et_big[:], d_ET.rearrange("(k p) d -> p k d", p=P))
            ET_s = [et_big[:, k, :] for k in range(8)]
            ones_blk = cp.tile([P, P], F32, tag="ones_blk", name="ones_blk")
            nc.vector.memset(ones_blk[:], 1.0)


            ones_row = cp.tile([1, P], F32, tag="ones_row", name="ones_row")
            nc.vector.memset(ones_row[:], 1.0)
            zero_col = cp.tile([P, 1], F32, tag="zero_col", name="zero_col")
            nc.vector.memset(zero_col[:], 0.0)
            eps_col = cp.tile([P, 1], F32, tag="eps_col", name="eps_col")
            nc.vector.memset(eps_col[:], LN_EPS)

            BF = mybir.dt.bfloat16
            ones_bfA = cp.tile([P, P], BF, tag="ones_bfA", name="ones_bfA")
            nc.vector.memset(ones_bfA[:], 1.0)
            ones_bfB = cp.tile([P, T], BF, tag="ones_bfB", name="ones_bfB")
            nc.vector.memset(ones_bfB[:], 1.0)
            warm_ps = pss.tile([P, T], F32, tag="pss", name="warm_ps")

            def warm(n):
                for _ in range(n):
                    nc.tensor.matmul(warm_ps[:], ones_bfA[:], ones_bfB[:],
                                     start=True, stop=True)

            warm(10)

            # ---- U = relu(emb @ Dx.T), row sums a ---------------------------
            U_s = [wp.tile([P, N], FDT, tag=f"U{m}", name=f"U{m}") for m in range(2)]
            a_s = [wp.tile([P, 1], F32, tag=f"a{m}", name=f"a{m}") for m in range(2)]
            apart = [[wp.tile([P, 1], F32, tag=f"ap{m}{c}", name=f"ap{m}{c}") for c in range(2)]
                     for m in range(2)]
            for mt in range(2):
                for ch in range(2):
                    pu = ps512.tile([P, 512], F32, tag="pu", name="pu")
                    for k in range(2):
                        _mm(nc, pu[:], embT_s[k][:, mt * P:(mt + 1) * P],
                            DxT_s[k][:, ch * 512:(ch + 1) * 512],
                            start=(k == 0), stop=(k == 1), fast=use_f32r)
                    nc.scalar.activation(
                        out=U_s[mt][:, ch * 512:(ch + 1) * 512], in_=pu[:],
                        func=AF.Relu, bias=zero_col[:], accum_out=apart[mt][ch][:])
                nc.vector.tensor_add(a_s[mt][:], apart[mt][0][:], apart[mt][1][:])

            # ---- scalar chain: b, log b, cumsum, p, q ------------------------
            logb_s = []
            q_s = []
            p_s = []
            for mt in range(2):
                bvec = wp.tile([P, 1], F32, tag=f"b{mt}", name=f"b{mt}")
                nc.vector.tensor_add(bvec[:], a_s[mt][:], c097_s[mt][:])
                lb = wp.tile([P, 1], F32, tag=f"lb{mt}", name=f"lb{mt}")
                nc.scalar.activation(out=lb[:], in_=bvec[:], func=AF.Ln, bias=zero_col[:])
                logb_s.append(lb)
            for mt in range(2):
                pl = pss.tile([P, 1], F32, tag="pss", name="plam")
                if mt == 0:
                    nc.tensor.matmul(pl[:], triu_s[:], logb_s[0][:],
                                     start=True, stop=True)
                else:
                    nc.tensor.matmul(pl[:], ones_blk[:], logb_s[0][:],
                                     start=True, stop=False)
                    nc.tensor.matmul(pl[:], triu_s[:], logb_s[1][:],
                                     start=False, stop=True)
                # q = lamS + iotaQ ; p = (iotaP - lamS) - logb   (lamS in PSUM)
                qv = wp.tile([P, 1], F32, tag=f"q{mt}", name=f"q{mt}")
                nc.vector.tensor_add(qv[:], pl[:], iotaQ_s[mt][:])
                q_s.append(qv)
                pv = wp.tile([P, 1], F32, tag=f"p{mt}", name=f"p{mt}")
                nc.vector.scalar_tensor_tensor(
                    out=pv[:], in0=iotaP_s[mt][:], scalar=pl[:],
                    in1=logb_s[mt][:], op0=ALU.subtract, op1=ALU.subtract)
                p_s.append(pv)

            warm(8)

            # ---- p as a row vector (PE transpose) ---------------------------
            p_row = wp.tile([1, T], F32, tag="p_row", name="p_row")
            for mt in range(2):
                pt = pss.tile([1, P], F32, tag="pss", name="ptr")
                nc.tensor.transpose(pt[:], p_s[mt][:], ident_s[:])
                nc.vector.tensor_copy(p_row[:, mt * P:(mt + 1) * P], pt[:])

            # ---- CT[s,t] = exp(q_s + p_t + mask) ----------------------------
            CT_s = []
            for st in range(2):
                pb = ps256.tile([P, T], F32, tag="ps", name="pb")
                nc.tensor.matmul(pb[:], ones_row[:], p_row[:],
                                 start=True, stop=True)
                tmp = wp.tile([P, T], F32, tag=f"ctmp{st}", name=f"ctmp{st}")
                nc.vector.tensor_add(tmp[:], pb[:], maskCT_s[st][:])
                ct = wp.tile([P, T], FDT, tag=f"CT{st}", name=f"CT{st}")
                nc.scalar.activation(out=ct[:], in_=tmp[:], func=AF.Exp,
                                     bias=q_s[st][:], scale=1.0)
                CT_s.append(ct)

            # ---- X^T = U^T C^T  (n on partitions, T free) -------------------
            XT_s = []
            for m in range(8):
                px = ps256.tile([P, T], F32, tag="ps", name="px")
                for k in range(2):
                    _mm(nc, px[:], U_s[k][:, m * P:(m + 1) * P], CT_s[k][:],
                        start=(k == 0), stop=(k == 1), fast=use_f32r)
                xt = wp.tile([P, T], FDT, tag=f"XT{m}", name=f"XT{m}")
                if m % 2 == 0:
                    nc.vector.tensor_copy(xt[:], px[:])
                else:
                    nc.scalar.copy(xt[:], px[:])
                XT_s.append(xt)

            # ---- W = ln(emb rows) -------------------------------------------
            W_s = []
            for mt in range(2):
                st6 = wp.tile([P, 6], F32, tag=f"wst{mt}", name=f"wst{mt}")
                nc.vector.bn_stats(st6[:], emb_s[mt][:])
                mv = wp.tile([P, 2], F32, tag=f"wmv{mt}", name=f"wmv{mt}")
                nc.vector.bn_aggr(mv[:], st6[:])
                lv = wp.tile([P, 1], F32, tag=f"wlv{mt}", name=f"wlv{mt}")
                nc.scalar.activation(out=lv[:], in_=mv[:, 1:2], func=AF.Ln,
                                     bias=eps_col[:])
                rs = wp.tile([P, 1], F32, tag=f"wrs{mt}", name=f"wrs{mt}")
                nc.scalar.activation(out=rs[:], in_=lv[:], func=AF.Exp,
                                     bias=zero_col[:], scale=-0.5)
                w = wp.tile([P, D], FDT, tag=f"W{mt}", name=f"W{mt}")
                nc.vector.tensor_scalar(w[:], emb_s[mt][:], mv[:, 0:1], rs[:],
                                        op0=ALU.subtract, op1=ALU.mult)
                W_s.append(w)

            # ---- G = X X^T ; GD = G o Dup -----------------------------------
            GD_s = []
            for st in range(2):
                pg = ps256.tile([P, T], F32, tag="ps", name="pg")
                for k in range(8):
                    _mm(nc, pg[:], XT_s[k][:, st * P:(st + 1) * P], XT_s[k][:],
                        start=(k == 0), stop=(k == 7), fast=use_f32r)
                gd = wp.tile([P, T], FDT, tag=f"GD{st}", name=f"GD{st}")
                nc.vector.tensor_mul(gd[:], pg[:], DupT_s[st][:])
                GD_s.append(gd)

            # ---- A = (G o D) @ W  ([t, d]) + layernorm ----------------------
            Aln_s = []
            for mt in range(2):
                pa = ps256.tile([P, D], F32, tag="ps", name="pa")
                ks = [0] if mt == 0 else [0, 1]
                for k in ks:
                    _mm(nc, pa[:], GD_s[k][:, mt * P:(mt + 1) * P], W_s[k][:],
                        start=(k == ks[0]), stop=(k == ks[-1]), fast=use_f32r)
                st6 = wp.tile([P, 6], F32, tag=f"ast{mt}", name=f"ast{mt}")
                nc.vector.bn_stats(st6[:], pa[:])
                mv = wp.tile([P, 2], F32, tag=f"amv{mt}", name=f"amv{mt}")
                nc.vector.bn_aggr(mv[:], st6[:])
                lv = wp.tile([P, 1], F32, tag=f"alv{mt}", name=f"alv{mt}")
                nc.scalar.activation(out=lv[:], in_=mv[:, 1:2], func=AF.Ln,
                                     bias=eps_col[:])
                rs = wp.tile([P, 1], F32, tag=f"ars{mt}", name=f"ars{mt}")
                nc.scalar.activation(out=rs[:], in_=lv[:], func=AF.Exp,
                                     bias=zero_col[:], scale=-0.5)
                al = wp.tile([P, D], F32, tag=f"Aln{mt}", name=f"Aln{mt}")
                nc.vector.tensor_scalar(al[:], pa[:], mv[:, 0:1], rs[:],
                                        op0=ALU.subtract, op1=ALU.mult)
                Aln_s.append(al)

            # ---- Aln^T via PE transpose ([d, t]) ----------------------------
            AlnT_s = [wp.tile([P, T], FDT, tag=f"AlnT{k}", name=f"AlnT{k}") for k in range(2)]
            for mt in range(2):
                for dt_ in range(2):
                    ptr = ps256.tile([P, P], F32, tag="ps", name="atr")
                    nc.tensor.transpose(ptr[:], Aln_s[mt][:, dt_ * P:(dt_ + 1) * P],
                                        ident_s[:])
                    nc.vector.tensor_copy(
                        AlnT_s[dt_][:, mt * P:(mt + 1) * P], ptr[:])

            # ---- y^T = relu(Dy ln(A)^T) o X^T -------------------------------
            yT_s = []
            for m in range(8):
                py = ps256.tile([P, T], F32, tag="ps", name="py")
                for k in range(2):
                    _mm(nc, py[:], DyT_s[k][:, m * P:(m + 1) * P], AlnT_s[k][:],
                        start=(k == 0), stop=(k == 1), fast=use_f32r)
                yt = wp.tile([P, T], FDT, tag=f"yT{m}", name=f"yT{m}")
                nc.vector.scalar_tensor_tensor(
                    out=yt[:], in0=py[:], scalar=0.0, in1=XT_s[m][:].bitcast(F32),
                    op0=ALU.max, op1=ALU.mult)
                yT_s.append(yt)

            # ---- v = y E^T ([t, d]) + layernorm + store ---------------------
            for mt in range(2):
                pv = ps256.tile([P, D], F32, tag="ps", name="pv")
                for k in range(8):
                    _mm(nc, pv[:], yT_s[k][:, mt * P:(mt + 1) * P], ET_s[k][:],
                        start=(k == 0), stop=(k == 7), fast=use_f32r)
                st6 = wp.tile([P, 6], F32, tag=f"ost{mt}", name=f"ost{mt}")
                nc.vector.bn_stats(st6[:], pv[:])
                mv = wp.tile([P, 2], F32, tag=f"omv{mt}", name=f"omv{mt}")
                nc.vector.bn_aggr(mv[:], st6[:])
                lv = wp.tile([P, 1], F32, tag=f"olv{mt}", name=f"olv{mt}")
                nc.scalar.activation(out=lv[:], in_=mv[:, 1:2], func=AF.Ln,
                                     bias=eps_col[:])
                rs = wp.tile([P, 1], F32, tag=f"ors{mt}", name=f"ors{mt}")
                nc.scalar.activation(out=rs[:], in_=lv[:], func=AF.Exp,
                                     bias=zero_col[:], scale=-0.5)
                ov = wp.tile([P, D], F32, tag=f"ov{mt}", name=f"ov{mt}")
                nc.vector.tensor_scalar(ov[:], pv[:], mv[:, 0:1], rs[:],
                                        op0=ALU.subtract, op1=ALU.mult)
                nc.sync.dma_start(d_out[mt * P:(mt + 1) * P, :], ov[:])

    nc.finalize()
    return nc


_NC_CACHE = {}


def _get_nc(use_f32r=USE_F32R):
    if use_f32r not in _NC_CACHE:
        _NC_CACHE[use_f32r] = _build_nc(use_f32r)
    return _NC_CACHE[use_f32r]


def _host_consts():
    ii = np.arange(T, dtype=np.float64)
    ln097 = np.log(np.float64(DECAY))
    maskCT = np.where(ii[:, None] <= ii[None, :], 0.0, -1e30).astype(np.float32)
    DupT = np.where(
        ii[:, None] < ii[None, :],
        np.float64(DECAY) ** (ii[None, :] - 1 - ii[:, None]),
        0.0,
    ).astype(np.float32)
    mask2 = np.ascontiguousarray(np.concatenate([maskCT, DupT], axis=1))
    tid = np.ascontiguousarray(np.concatenate(
        [np.triu(np.ones((P, P), np.float32), k=1), np.eye(P, dtype=np.float32)],
        axis=1))
    sc = np.zeros((T, 4), np.float32)
    sc[:, 0] = DECAY
    sc[0, 0] = 0.0
    sc[:, 1] = (ii * ln097).astype(np.float32)
    sc[:, 2] = (-ii * ln097).astype(np.float32)
    return sc, tid, mask2


def make_in_maps(embeddings, E, Dx, Dy):
    emb = np.ascontiguousarray(np.asarray(embeddings, dtype=np.float32))
    E = np.asarray(E, dtype=np.float32)
    Dx = np.asarray(Dx, dtype=np.float32)
    Dy = np.asarray(Dy, dtype=np.float32)
    sc, tid, mask2 = _host_consts()
    shared = {
        "sc": sc, "tid": tid, "mask2": mask2,
        "DxT": np.ascontiguousarray(Dx.T),
        "DyT": np.ascontiguousarray(Dy.T),
        "ET": np.ascontiguousarray(E.T),
    }
    in_maps = []
    for b in range(B):
        m = dict(shared)
        m["em2"] = np.ascontiguousarray(
            np.concatenate([emb[b], emb[b].T], axis=1))
        in_maps.append(m)
    return in_maps


def kernel(embeddings, E, Dx, Dy, _use_f32r=USE_F32R):
    in_maps = make_in_maps(embeddings, E, Dx, Dy)
    nc = _get_nc(_use_f32r)
    res = run_bass_kernel_spmd(nc, in_maps, core_ids=list(range(B)))
    return np.stack([r["out"] for r in res.results], axis=0)


# revision 18
# speedup vs baseline: 1.0664x; 1.0664x over previous
"""Trainium2 Bass kernel for the BDH recurrent block (B=8, T=256, d=256, n=1024).

Key reformulation: the scan input v_prev is the *embedding* at each step (the
output v_star is never fed back), so the only recurrences are

  x_t = l1norm(0.97 * x_{t-1} + relu(emb_t @ Dx.T))          (elementwise, n)
  rho_t = 0.97 * rho_{t-1} + ln(emb_t) (x) x_t               (rank-1, d*n)

Both have closed forms:
  x_t  = sum_s C[t,s] * U_s           with U = relu(emb @ Dx.T)  and
         C[t,s] = 0.97^{t-s} / prod_{r=s..t} b_r,  b_r = sum(U_r) + 0.97*[r>0]
         (b_0 = sum(U_0)), computed in log space via a cumulative sum.
  a*_t = rho_{t-1} x_t = sum_{s<t} 0.97^{t-1-s} (x_s . x_t) ln(emb_s)
       = ((X X^T) o D) @ ln(emb)     -- decay-masked attention.

So the whole T-step scan becomes a handful of dense matmuls, one sample per
NeuronCore (data-parallel over B=8 across 8 cores, weights replicated).
"""

import numpy as np

import concourse.bass as bass
import concourse.tile as tile
from concourse import bacc, mybir
from concourse.bass_utils import run_bass_kernel_spmd
from concourse.hw_specs import get_activation_tables

B, T, D, N = 8, 256, 256, 1024
P = 128  # partitions
LN_EPS = 1e-5
DECAY = 0.97
F32 = mybir.dt.float32
F32R = mybir.dt.float32r
AF = mybir.ActivationFunctionType
ALU = mybir.AluOpType

# fp32r runs the PE at 4x the fp32 rate (1 cycle/row at N>=256); inputs are
# fp32 in SBUF, bitcast at the matmul. Used only for the big matmuls.
USE_F32R = True


def _mm(nc, out, lhsT, rhs, start, stop, fast):
    nc.tensor.matmul(out, lhsT, rhs, start=start, stop=stop)


def _build_nc(use_f32r=USE_F32R):
    nc = bacc.Bacc()
    FDT = F32R if use_f32r else F32

    # packed inputs (few large DMAs, ordered by when the pipeline needs them)
    d_sc = nc.dram_tensor("sc", [T, 4], F32, kind="ExternalInput")  # c097|iotaP|iotaQ|0
    d_tid = nc.dram_tensor("tid", [P, 2 * P], F32, kind="ExternalInput")  # triu|ident
    d_em2 = nc.dram_tensor("em2", [T, 2 * D], FDT, kind="ExternalInput")  # emb|embT
    d_DxT = nc.dram_tensor("DxT", [D, N], FDT, kind="ExternalInput")
    d_mask2 = nc.dram_tensor("mask2", [T, 2 * T], F32, kind="ExternalInput")  # maskCT|DupT
    d_DyT = nc.dram_tensor("DyT", [D, N], FDT, kind="ExternalInput")
    d_ET = nc.dram_tensor("ET", [N, D], FDT, kind="ExternalInput")
    d_out = nc.dram_tensor("out", [T, D], F32, kind="ExternalOutput")

    # Preload the one ACT table set containing every function we use
    # (relu/ln/exp/copy) so the compiler never swaps tables mid-kernel
    # (each swap costs ~2.7us on the Scalar engine).
    act_sets = list(get_activation_tables(nc.m.arch))
    combined_set_id = act_sets.index("natural_log_exp_and_others")

    with tile.TileContext(nc) as tc:
        nc.scalar.add_instruction(mybir.InstLoadActFuncSet(
            name=nc.get_next_instruction_name(),
            act_func_set_id=combined_set_id, ins=[], outs=[]))
        with (
            tc.tile_pool(name="consts", bufs=1) as cp,
            tc.tile_pool(name="work", bufs=1) as wp,
            tc.tile_pool(name="ps512", bufs=2, space="PSUM") as ps512,
            tc.tile_pool(name="ps256", bufs=4, space="PSUM") as ps256,
            tc.tile_pool(name="ps_small", bufs=2, space="PSUM") as pss,
        ):
            # ---- load inputs (issue order == need order) --------------------
            def load2(dram, f, tag, dt_=F32):  # (2P, f) dram -> two [P, f] tiles
                ts = []
                for k in range(2):
                    t = cp.tile([P, f], dt_, tag=f"{tag}{k}", name=f"{tag}{k}")
                    nc.sync.dma_start(t[:], dram[k * P:(k + 1) * P, :])
                    ts.append(t)
                return ts

            sc_s = load2(d_sc, 4, "sc")
            c097_s = [t[:, 0:1] for t in sc_s]
            iotaP_s = [t[:, 1:2] for t in sc_s]
            iotaQ_s = [t[:, 2:3] for t in sc_s]
            tid_s = cp.tile([P, 2 * P], F32, tag="tid", name="tid")
            nc.sync.dma_start(tid_s[:], d_tid[:, :])
            triu_s = tid_s[:, 0:P]
            ident_s = tid_s[:, P:2 * P]
            em2_s = load2(d_em2, 2 * D, "em2", FDT)
            emb_s = [t[:, 0:D].bitcast(F32) for t in em2_s]
            embT_s = [t[:, D:2 * D] for t in em2_s]
            DxT_s = load2(d_DxT, N, "DxT", FDT)
            def load2g(dram, f, tag, dt_=F32):
                ts = []
                for k in range(2):
                    t = cp.tile([P, f], dt_, tag=f"{tag}{k}", name=f"{tag}{k}")
                    nc.gpsimd.dma_start(t[:], dram[k * P:(k + 1) * P, :])
                    ts.append(t)
                return ts

            mask2_s = load2(d_mask2, 2 * T, "mask2")
            maskCT_s = [t[:, 0:T] for t in mask2_s]
            DupT_s = [t[:, T:2 * T] for t in mask2_s]
            DyT_s = load2(d_DyT, N, "DyT", FDT)
            et_big = cp.tile([P, 8, D], FDT, tag="et_big", name="et_big")
            nc.sync.dma_start(
                et_big[:], d_ET.rearrange("(k p) d -> p k d", p=P))
            ET_s = [et_big[:, k, :] for k in range(8)]
            ones_blk = cp.tile([P, P], F32, tag="ones_blk", name="ones_blk")
            nc.vector.memset(ones_blk[:], 1.0)


            ones_row = cp.tile([1, P], F32, tag="ones_row", name="ones_row")
            nc.vector.memset(ones_row[:], 1.0)
            zero_col = cp.tile([P, 1], F32, tag="zero_col", name="zero_col")
            nc.vector.memset(zero_col[:], 0.0)
            eps_col = cp.tile([P, 1], F32, tag="eps_col", name="eps_col")
            nc.vector.memset(eps_col[:], LN_EPS)

            BF = mybir.dt.bfloat16
            ones_bfA = cp.tile([P, P], BF, tag="ones_bfA", name="ones_bfA")
            nc.vector.memset(ones_bfA[:], 1.0)
            ones_bfB = cp.tile([P, T], BF, tag="ones_bfB", name="ones_bfB")
            nc.vector.memset(ones_bfB[:], 1.0)
            warm_ps = pss.tile([P, T], F32, tag="pss", name="warm_ps")

            def warm(n):
                for _ in range(n):
                    nc.tensor.matmul(warm_ps[:], ones_bfA[:], ones_bfB[:],
                                     start=True, stop=True)

            warm(10)

            # ---- U = relu(emb @ Dx.T), row sums a ---------------------------
            U_s = [wp.tile([P, N], FDT, tag=f"U{m}", name=f"U{m}") for m in range(2)]
            a_s = [wp.tile([P, 1], F32, tag=f"a{m}", name=f"a{m}") for m in range(2)]
            apart = [[wp.tile([P, 1], F32, tag=f"ap{m}{c}", name=f"ap{m}{c}") for c in range(2)]
                     for m in range(2)]
            for mt in range(2):
                for ch in range(2):
                    pu = ps512.tile([P, 512], F32, tag="pu", name="pu")
                    for k in range(2):
                        _mm(nc, pu[:], embT_s[k][:, mt * P:(mt + 1) * P],
                            DxT_s[k][:, ch * 512:(ch + 1) * 512],
                            start=(k == 0), stop=(k == 1), fast=use_f32r)
                    nc.scalar.activation(
                        out=U_s[mt][:, ch * 512:(ch + 1) * 512], in_=pu[:],
                        func=AF.Relu, bias=zero_col[:], accum_out=apart[mt][ch][:])
                nc.vector.tensor_add(a_s[mt][:], apart[mt][0][:], apart[mt][1][:])

            # ---- scalar chain: b, log b, cumsum, p, q ------------------------
            logb_s = []
            q_s = []
            p_s = []
            for mt in range(2):
                bvec = wp.tile([P, 1], F32, tag=f"b{mt}", name=f"b{mt}")
                nc.vector.tensor_add(bvec[:], a_s[mt][:], c097_s[mt][:])
                lb = wp.tile([P, 1], F32, tag=f"lb{mt}", name=f"lb{mt}")
                nc.scalar.activation(out=lb[:], in_=bvec[:], func=AF.Ln, bias=zero_col[:])
                logb_s.append(lb)
            for mt in range(2):
                pl = pss.tile([P, 1], F32, tag="pss", name="plam")
                if mt == 0:
                    nc.tensor.matmul(pl[:], triu_s[:], logb_s[0][:],
                                     start=True, stop=True)
                else:
                    nc.tensor.matmul(pl[:], ones_blk[:], logb_s[0][:],
                                     start=True, stop=False)
                    nc.tensor.matmul(pl[:], triu_s[:], logb_s[1][:],
                                     start=False, stop=True)
                # q = lamS + iotaQ ; p = (iotaP - lamS) - logb   (lamS in PSUM)
                qv = wp.tile([P, 1], F32, tag=f"q{mt}", name=f"q{mt}")
                nc.vector.tensor_add(qv[:], pl[:], iotaQ_s[mt][:])
                q_s.append(qv)
                pv = wp.tile([P, 1], F32, tag=f"p{mt}", name=f"p{mt}")
                nc.vector.scalar_tensor_tensor(
                    out=pv[:], in0=iotaP_s[mt][:], scalar=pl[:],
                    in1=logb_s[mt][:], op0=ALU.subtract, op1=ALU.subtract)
                p_s.append(pv)

            warm(8)

            # ---- p as a row vector (PE transpose) ---------------------------
            p_row = wp.tile([1, T], F32, tag="p_row", name="p_row")
            for mt in range(2):
                pt = pss.tile([1, P], F32, tag="pss", name="ptr")
                nc.tensor.transpose(pt[:], p_s[mt][:], ident_s[:])
                nc.vector.tensor_copy(p_row[:, mt * P:(mt + 1) * P], pt[:])

            # ---- CT[s,t] = exp(q_s + p_t + mask) ----------------------------
            CT_s = []
            for st in range(2):
                pb = ps256.tile([P, T], F32, tag="ps", name="pb")
                nc.tensor.matmul(pb[:], ones_row[:], p_row[:],
                                 start=True, stop=True)
                tmp = wp.tile([P, T], F32, tag=f"ctmp{st}", name=f"ctmp{st}")
                nc.vector.tensor_add(tmp[:], pb[:], maskCT_s[st][:])
                ct = wp.tile([P, T], FDT, tag=f"CT{st}", name=f"CT{st}")
                nc.scalar.activation(out=ct[:], in_=tmp[:], func=AF.Exp,
                                     bias=q_s[st][:], scale=1.0)
                CT_s.append(ct)

            # ---- X^T = U^T C^T  (n on partitions, T free) -------------------
            XT_s = []
            for m in range(8):
                px = ps256.tile([P, T], F32, tag="ps", name="px")
                for k in range(2):
                    _mm(nc, px[:], U_s[k][:, m * P:(m + 1) * P], CT_s[k][:],
                        start=(k == 0), stop=(k == 1), fast=use_f32r)
                xt = wp.tile([P, T], FDT, tag=f"XT{m}", name=f"XT{m}")
                if m % 2 == 0:
                    nc.vector.tensor_copy(xt[:], px[:])
                else:
                    nc.scalar.copy(xt[:], px[:])
                XT_s.append(xt)

            # ---- W = ln(emb rows) -------------------------------------------
            W_s = []
            for mt in range(2):
                st6 = wp.tile([P, 6], F32, tag=f"wst{mt}", name=f"wst{mt}")
                nc.vector.bn_stats(st6[:], emb_s[mt][:])
                mv = wp.tile([P, 2], F32, tag=f"wmv{mt}", name=f"wmv{mt}")
                nc.vector.bn_aggr(mv[:], st6[:])
                lv = wp.tile([P, 1], F32, tag=f"wlv{mt}", name=f"wlv{mt}")
                nc.scalar.activation(out=lv[:], in_=mv[:, 1:2], func=AF.Ln,
                                     bias=eps_col[:])
                rs = wp.tile([P, 1], F32, tag=f"wrs{mt}", name=f"wrs{mt}")
                nc.scalar.activation(out=rs[:], in_=lv[:], func=AF.Exp,
                                     bias=zero_col[:], scale=-0.5)
                w = wp.tile([P, D], FDT, tag=f"W{mt}", name=f"W{mt}")
                nc.vector.tensor_scalar(w[:], emb_s[mt][:], mv[:, 0:1], rs[:],
                                        op0=ALU.subtract, op1=ALU.mult)
                W_s.append(w)

            # ---- G = X X^T ; GD = G o Dup -----------------------------------
            GD_s = []
            for st in range(2):
                pg = ps256.tile([P, T], F32, tag="ps", name="pg")
                for k in range(8):
                    _mm(nc, pg[:], XT_s[k][:, st * P:(st + 1) * P], XT_s[k][:],
                        start=(k == 0), stop=(k == 7), fast=use_f32r)
                gd = wp.tile([P, T], FDT, tag=f"GD{st}", name=f"GD{st}")
                nc.vector.tensor_mul(gd[:], pg[:], DupT_s[st][:])
                GD_s.append(gd)

            # ---- A = (G o D) @ W  ([t, d]) + layernorm ----------------------
            Aln_s = []
            for mt in range(2):
                pa = ps256.tile([P, D], F32, tag="ps", name="pa")
                ks = [0] if mt == 0 else [0, 1]
                for k in ks:
                    _mm(nc, pa[:], GD_s[k][:, mt * P:(mt + 1) * P], W_s[k][:],
                        start=(k == ks[0]), stop=(k == ks[-1]), fast=use_f32r)
                st6 = wp.tile([P, 6], F32, tag=f"ast{mt}", name=f"ast{mt}")
                nc.vector.bn_stats(st6[:], pa[:])
                mv = wp.tile([P, 2], F32, tag=f"amv{mt}", name=f"amv{mt}")
                nc.vector.bn_aggr(mv[:], st6[:])
                lv = wp.tile([P, 1], F32, tag=f"alv{mt}", name=f"alv{mt}")
                nc.scalar.activation(out=lv[:], in_=mv[:, 1:2], func=AF.Ln,
                                     bias=eps_col[:])
                rs = wp.tile([P, 1], F32, tag=f"ars{mt}", name=f"ars{mt}")
                nc.scalar.activation(out=rs[:], in_=lv[:], func=AF.Exp,
                                     bias=zero_col[:], scale=-0.5)
                al = wp.tile([P, D], F32, tag=f"Aln{mt}", name=f"Aln{mt}")
                nc.vector.tensor_scalar(al[:], pa[:], mv[:, 0:1], rs[:],
                                        op0=ALU.subtract, op1=ALU.mult)
                Aln_s.append(al)

            # ---- Aln^T via PE transpose ([d, t]) ----------------------------
            AlnT_s = [wp.tile([P, T], FDT, tag=f"AlnT{k}", name=f"AlnT{k}") for k in range(2)]
            for mt in range(2):
                for dt_ in range(2):
                    ptr = ps256.tile([P, P], F32, tag="ps", name="atr")
                    nc.tensor.transpose(ptr[:], Aln_s[mt][:, dt_ * P:(dt_ + 1) * P],
                                        ident_s[:])
                    nc.vector.tensor_copy(
                        AlnT_s[dt_][:, mt * P:(mt + 1) * P], ptr[:])

            # ---- y^T = relu(Dy ln(A)^T) o X^T -------------------------------
            yT_s = []
            for m in range(8):
                py = ps256.tile([P, T], F32, tag="ps", name="py")
                for k in range(2):
                    _mm(nc, py[:], DyT_s[k][:, m * P:(m + 1) * P], AlnT_s[k][:],
                        start=(k == 0), stop=(k == 1), fast=use_f32r)
                yt = wp.tile([P, T], FDT, tag=f"yT{m}", name=f"yT{m}")
                nc.vector.scalar_tensor_tensor(
                    out=yt[:], in0=py[:], scalar=0.0, in1=XT_s[m][:].bitcast(F32),
                    op0=ALU.max, op1=ALU.mult)
                yT_s.append(yt)

            # ---- v = y E^T ([t, d]) + layernorm + store ---------------------
            for mt in range(2):
                pv = ps256.tile([P, D], F32, tag="ps", name="pv")
                for k in range(8):
                    _mm(nc, pv[:], yT_s[k][:, mt * P:(mt + 1) * P], ET_s[k][:],
                        start=(k == 0), stop=(k == 7), fast=use_f32r)
                st6 = wp.tile([P, 6], F32, tag=f"ost{mt}", name=f"ost{mt}")
                nc.vector.bn_stats(st6[:], pv[:])
                mv = wp.tile([P, 2], F32, tag=f"omv{mt}", name=f"omv{mt}")
                nc.vector.bn_aggr(mv[:], st6[:])
                lv = wp.tile([P, 1], F32, tag=f"olv{mt}", name=f"olv{mt}")
                nc.scalar.activation(out=lv[:], in_=mv[:, 1:2], func=AF.Ln,
                                     bias=eps_col[:])
                rs = wp.tile([P, 1], F32, tag=f"ors{mt}", name=f"ors{mt}")
                nc.scalar.activation(out=rs[:], in_=lv[:], func=AF.Exp,
                                     bias=zero_col[:], scale=-0.5)
                ov = wp.tile([P, D], F32, tag=f"ov{mt}", name=f"ov{mt}")
                nc.vector.tensor_scalar(ov[:], pv[:], mv[:, 0:1], rs[:],
                                        op0=ALU.subtract, op1=ALU.mult)
                nc.sync.dma_start(d_out[mt * P:(mt + 1) * P, :], ov[:])

    nc.finalize()
    return nc


_NC_CACHE = {}


def _get_nc(use_f32r=USE_F32R):
    if use_f32r not in _NC_CACHE:
        _NC_CACHE[use_f32r] = _build_nc(use_f32r)
    return _NC_CACHE[use_f32r]


def _host_consts():
    ii = np.arange(T, dtype=np.float64)
    ln097 = np.log(np.float64(DECAY))
    maskCT = np.where(ii[:, None] <= ii[None, :], 0.0, -1e30).astype(np.float32)
    DupT = np.where(
        ii[:, None] < ii[None, :],
        np.float64(DECAY) ** (ii[None, :] - 1 - ii[:, None]),
        0.0,
    ).astype(np.float32)
    mask2 = np.ascontiguousarray(np.concatenate([maskCT, DupT], axis=1))
    tid = np.ascontiguousarray(np.concatenate(
        [np.triu(np.ones((P, P), np.float32), k=1), np.eye(P, dtype=np.float32)],
        axis=1))
    sc = np.zeros((T, 4), np.float32)
    sc[:, 0] = DECAY
    sc[0, 0] = 0.0
    sc[:, 1] = (ii * ln097).astype(np.float32)
    sc[:, 2] = (-ii * ln097).astype(np.float32)
    return sc, tid, mask2


def make_in_maps(embeddings, E, Dx, Dy):
    emb = np.ascontiguousarray(np.asarray(embeddings, dtype=np.float32))
    E = np.asarray(E, dtype=np.float32)
    Dx = np.asarray(Dx, dtype=np.float32)
    Dy = np.asarray(Dy, dtype=np.float32)
    sc, tid, mask2 = _host_consts()
    shared = {
        "sc": sc, "tid": tid, "mask2": mask2,
        "DxT": np.ascontiguousarray(Dx.T),
        "DyT": np.ascontiguousarray(Dy.T),
        "ET": np.ascontiguousarray(E.T),
    }
    in_maps = []
    for b in range(B):
        m = dict(shared)
        m["em2"] = np.ascontiguousarray(
            np.concatenate([emb[b], emb[b].T], axis=1))
        in_maps.append(m)
    return in_maps


def kernel(embeddings, E, Dx, Dy, _use_f32r=USE_F32R):
    in_maps = make_in_maps(embeddings, E, Dx, Dy)
    nc = _get_nc(_use_f32r)
    res = run_bass_kernel_spmd(nc, in_maps, core_ids=list(range(B)))
    return np.stack([r["out"] for r in res.results], axis=0)


# revision 19
# speedup vs baseline: 1.1290x; 1.0587x over previous
"""Trainium2 Bass kernel for the BDH recurrent block (B=8, T=256, d=256, n=1024).

Key reformulation: the scan input v_prev is the *embedding* at each step (the
output v_star is never fed back), so the only recurrences are

  x_t = l1norm(0.97 * x_{t-1} + relu(emb_t @ Dx.T))          (elementwise, n)
  rho_t = 0.97 * rho_{t-1} + ln(emb_t) (x) x_t               (rank-1, d*n)

Both have closed forms:
  x_t  = sum_s C[t,s] * U_s           with U = relu(emb @ Dx.T)  and
         C[t,s] = 0.97^{t-s} / prod_{r=s..t} b_r,  b_r = sum(U_r) + 0.97*[r>0]
         (b_0 = sum(U_0)), computed in log space via a cumulative sum.
  a*_t = rho_{t-1} x_t = sum_{s<t} 0.97^{t-1-s} (x_s . x_t) ln(emb_s)
       = ((X X^T) o D) @ ln(emb)     -- decay-masked attention.

So the whole T-step scan becomes a handful of dense matmuls, one sample per
NeuronCore (data-parallel over B=8 across 8 cores, weights replicated).
"""

import numpy as np

import concourse.bass as bass
import concourse.tile as tile
from concourse import bacc, mybir
from concourse.bass_utils import run_bass_kernel_spmd
from concourse.hw_specs import get_activation_tables

B, T, D, N = 8, 256, 256, 1024
P = 128  # partitions
LN_EPS = 1e-5
DECAY = 0.97
F32 = mybir.dt.float32
F32R = mybir.dt.float32r
AF = mybir.ActivationFunctionType
ALU = mybir.AluOpType

# fp32r runs the PE at 4x the fp32 rate (1 cycle/row at N>=256); inputs are
# fp32 in SBUF, bitcast at the matmul. Used only for the big matmuls.
USE_F32R = True


def _mm(nc, out, lhsT, rhs, start, stop, fast):
    nc.tensor.matmul(out, lhsT, rhs, start=start, stop=stop)


def _build_nc(use_f32r=USE_F32R):
    nc = bacc.Bacc()
    FDT = F32R if use_f32r else F32

    # packed inputs (few large DMAs, ordered by when the pipeline needs them)
    d_sc = nc.dram_tensor("sc", [T, 4], F32, kind="ExternalInput")  # c097|iotaP|iotaQ|0
    d_tid = nc.dram_tensor("tid", [P, 2 * P], F32, kind="ExternalInput")  # triu|ident
    d_em2 = nc.dram_tensor("em2", [T, 2 * D], FDT, kind="ExternalInput")  # emb|embT
    d_DxT = nc.dram_tensor("DxT", [D, N], FDT, kind="ExternalInput")
    d_mask2 = nc.dram_tensor("mask2", [T, 2 * T], F32, kind="ExternalInput")  # maskCT|DupT
    d_DyT = nc.dram_tensor("DyT", [D, N], FDT, kind="ExternalInput")
    d_ET = nc.dram_tensor("ET", [N, D], FDT, kind="ExternalInput")
    d_out = nc.dram_tensor("out", [T, D], F32, kind="ExternalOutput")

    # Preload the one ACT table set containing every function we use
    # (relu/ln/exp/copy) so the compiler never swaps tables mid-kernel
    # (each swap costs ~2.7us on the Scalar engine).
    act_sets = list(get_activation_tables(nc.m.arch))
    combined_set_id = act_sets.index("natural_log_exp_and_others")

    with tile.TileContext(nc) as tc:
        nc.scalar.add_instruction(mybir.InstLoadActFuncSet(
            name=nc.get_next_instruction_name(),
            act_func_set_id=combined_set_id, ins=[], outs=[]))
        with (
            tc.tile_pool(name="consts", bufs=1) as cp,
            tc.tile_pool(name="work", bufs=1) as wp,
            tc.tile_pool(name="ps512", bufs=2, space="PSUM") as ps512,
            tc.tile_pool(name="ps256", bufs=4, space="PSUM") as ps256,
            tc.tile_pool(name="ps_small", bufs=2, space="PSUM") as pss,
        ):
            # ---- load inputs (issue order == need order) --------------------
            def load2(dram, f, tag, dt_=F32):  # (2P, f) dram -> two [P, f] tiles
                ts = []
                for k in range(2):
                    t = cp.tile([P, f], dt_, tag=f"{tag}{k}", name=f"{tag}{k}")
                    nc.sync.dma_start(t[:], dram[k * P:(k + 1) * P, :])
                    ts.append(t)
                return ts

            sc_s = load2(d_sc, 4, "sc")
            c097_s = [t[:, 0:1] for t in sc_s]
            iotaP_s = [t[:, 1:2] for t in sc_s]
            iotaQ_s = [t[:, 2:3] for t in sc_s]
            tid_s = cp.tile([P, 2 * P], F32, tag="tid", name="tid")
            nc.sync.dma_start(tid_s[:], d_tid[:, :])
            triu_s = tid_s[:, 0:P]
            ident_s = tid_s[:, P:2 * P]
            em2_s = load2(d_em2, 2 * D, "em2", FDT)
            emb_s = [t[:, 0:D].bitcast(F32) for t in em2_s]
            embT_s = [t[:, D:2 * D] for t in em2_s]
            DxT_s = load2(d_DxT, N, "DxT", FDT)
            def load2g(dram, f, tag, dt_=F32):
                ts = []
                for k in range(2):
                    t = cp.tile([P, f], dt_, tag=f"{tag}{k}", name=f"{tag}{k}")
                    nc.gpsimd.dma_start(t[:], dram[k * P:(k + 1) * P, :])
                    ts.append(t)
                return ts

            mask2_s = load2(d_mask2, 2 * T, "mask2")
            maskCT_s = [t[:, 0:T] for t in mask2_s]
            DupT_s = [t[:, T:2 * T] for t in mask2_s]
            DyT_s = load2(d_DyT, N, "DyT", FDT)
            et_big = cp.tile([P, 8, D], FDT, tag="et_big", name="et_big")
            nc.sync.dma_start(
                et_big[:], d_ET.rearrange("(k p) d -> p k d", p=P))
            ET_s = [et_big[:, k, :] for k in range(8)]
            ones_blk = cp.tile([P, P], F32, tag="ones_blk", name="ones_blk")
            nc.vector.memset(ones_blk[:], 1.0)


            ones_row = cp.tile([1, P], F32, tag="ones_row", name="ones_row")
            nc.vector.memset(ones_row[:], 1.0)
            zero_col = cp.tile([P, 1], F32, tag="zero_col", name="zero_col")
            nc.vector.memset(zero_col[:], 0.0)
            eps_col = cp.tile([P, 1], F32, tag="eps_col", name="eps_col")
            nc.vector.memset(eps_col[:], LN_EPS)

            # ---- U = relu(emb @ Dx.T), row sums a ---------------------------
            U_s = [wp.tile([P, N], FDT, tag=f"U{m}", name=f"U{m}") for m in range(2)]
            a_s = [wp.tile([P, 1], F32, tag=f"a{m}", name=f"a{m}") for m in range(2)]
            apart = [[wp.tile([P, 1], F32, tag=f"ap{m}{c}", name=f"ap{m}{c}") for c in range(2)]
                     for m in range(2)]
            for mt in range(2):
                for ch in range(2):
                    pu = ps512.tile([P, 512], F32, tag="pu", name="pu")
                    for k in range(2):
                        _mm(nc, pu[:], embT_s[k][:, mt * P:(mt + 1) * P],
                            DxT_s[k][:, ch * 512:(ch + 1) * 512],
                            start=(k == 0), stop=(k == 1), fast=use_f32r)
                    nc.scalar.activation(
                        out=U_s[mt][:, ch * 512:(ch + 1) * 512], in_=pu[:],
                        func=AF.Relu, bias=zero_col[:], accum_out=apart[mt][ch][:])
                nc.vector.tensor_add(a_s[mt][:], apart[mt][0][:], apart[mt][1][:])

            # ---- scalar chain: b, log b, cumsum, p, q ------------------------
            logb_s = []
            q_s = []
            p_s = []
            for mt in range(2):
                bvec = wp.tile([P, 1], F32, tag=f"b{mt}", name=f"b{mt}")
                nc.vector.tensor_add(bvec[:], a_s[mt][:], c097_s[mt][:])
                lb = wp.tile([P, 1], F32, tag=f"lb{mt}", name=f"lb{mt}")
                nc.scalar.activation(out=lb[:], in_=bvec[:], func=AF.Ln, bias=zero_col[:])
                logb_s.append(lb)
            for mt in range(2):
                pl = pss.tile([P, 1], F32, tag="pss", name="plam")
                if mt == 0:
                    nc.tensor.matmul(pl[:], triu_s[:], logb_s[0][:],
                                     start=True, stop=True)
                else:
                    nc.tensor.matmul(pl[:], ones_blk[:], logb_s[0][:],
                                     start=True, stop=False)
                    nc.tensor.matmul(pl[:], triu_s[:], logb_s[1][:],
                                     start=False, stop=True)
                # q = lamS + iotaQ ; p = (iotaP - lamS) - logb   (lamS in PSUM)
                qv = wp.tile([P, 1], F32, tag=f"q{mt}", name=f"q{mt}")
                nc.vector.tensor_add(qv[:], pl[:], iotaQ_s[mt][:])
                q_s.append(qv)
                pv = wp.tile([P, 1], F32, tag=f"p{mt}", name=f"p{mt}")
                nc.vector.scalar_tensor_tensor(
                    out=pv[:], in0=iotaP_s[mt][:], scalar=pl[:],
                    in1=logb_s[mt][:], op0=ALU.subtract, op1=ALU.subtract)
                p_s.append(pv)

            # ---- p as a row vector (PE transpose) ---------------------------
            p_row = wp.tile([1, T], F32, tag="p_row", name="p_row")
            for mt in range(2):
                pt = pss.tile([1, P], F32, tag="pss", name="ptr")
                nc.tensor.transpose(pt[:], p_s[mt][:], ident_s[:])
                nc.vector.tensor_copy(p_row[:, mt * P:(mt + 1) * P], pt[:])

            # ---- CT[s,t] = exp(q_s + p_t + mask) ----------------------------
            CT_s = []
            for st in range(2):
                pb = ps256.tile([P, T], F32, tag="ps", name="pb")
                nc.tensor.matmul(pb[:], ones_row[:], p_row[:],
                                 start=True, stop=True)
                tmp = wp.tile([P, T], F32, tag=f"ctmp{st}", name=f"ctmp{st}")
                nc.vector.tensor_add(tmp[:], pb[:], maskCT_s[st][:])
                ct = wp.tile([P, T], FDT, tag=f"CT{st}", name=f"CT{st}")
                nc.scalar.activation(out=ct[:], in_=tmp[:], func=AF.Exp,
                                     bias=q_s[st][:], scale=1.0)
                CT_s.append(ct)

            # ---- X^T = U^T C^T  (n on partitions, T free) -------------------
            XT_s = []
            for m in range(8):
                px = ps256.tile([P, T], F32, tag="ps", name="px")
                for k in range(2):
                    _mm(nc, px[:], U_s[k][:, m * P:(m + 1) * P], CT_s[k][:],
                        start=(k == 0), stop=(k == 1), fast=use_f32r)
                xt = wp.tile([P, T], FDT, tag=f"XT{m}", name=f"XT{m}")
                if m % 2 == 0:
                    nc.vector.tensor_copy(xt[:], px[:])
                else:
                    nc.scalar.copy(xt[:], px[:])
                XT_s.append(xt)

            # ---- W = ln(emb rows) -------------------------------------------
            W_s = []
            for mt in range(2):
                st6 = wp.tile([P, 6], F32, tag=f"wst{mt}", name=f"wst{mt}")
                nc.vector.bn_stats(st6[:], emb_s[mt][:])
                mv = wp.tile([P, 2], F32, tag=f"wmv{mt}", name=f"wmv{mt}")
                nc.vector.bn_aggr(mv[:], st6[:])
                lv = wp.tile([P, 1], F32, tag=f"wlv{mt}", name=f"wlv{mt}")
                nc.scalar.activation(out=lv[:], in_=mv[:, 1:2], func=AF.Ln,
                                     bias=eps_col[:])
                rs = wp.tile([P, 1], F32, tag=f"wrs{mt}", name=f"wrs{mt}")
                nc.scalar.activation(out=rs[:], in_=lv[:], func=AF.Exp,
                                     bias=zero_col[:], scale=-0.5)
                w = wp.tile([P, D], FDT, tag=f"W{mt}", name=f"W{mt}")
                nc.vector.tensor_scalar(w[:], emb_s[mt][:], mv[:, 0:1], rs[:],
                                        op0=ALU.subtract, op1=ALU.mult)
                W_s.append(w)

            # ---- G = X X^T ; GD = G o Dup -----------------------------------
            GD_s = []
            for st in range(2):
                pg = ps256.tile([P, T], F32, tag="ps", name="pg")
                for k in range(8):
                    _mm(nc, pg[:], XT_s[k][:, st * P:(st + 1) * P], XT_s[k][:],
                        start=(k == 0), stop=(k == 7), fast=use_f32r)
                gd = wp.tile([P, T], FDT, tag=f"GD{st}", name=f"GD{st}")
                nc.vector.tensor_mul(gd[:], pg[:], DupT_s[st][:])
                GD_s.append(gd)

            # ---- A = (G o D) @ W  ([t, d]) + layernorm ----------------------
            Aln_s = []
            for mt in range(2):
                pa = ps256.tile([P, D], F32, tag="ps", name="pa")
                ks = [0] if mt == 0 else [0, 1]
                for k in ks:
                    _mm(nc, pa[:], GD_s[k][:, mt * P:(mt + 1) * P], W_s[k][:],
                        start=(k == ks[0]), stop=(k == ks[-1]), fast=use_f32r)
                st6 = wp.tile([P, 6], F32, tag=f"ast{mt}", name=f"ast{mt}")
                nc.vector.bn_stats(st6[:], pa[:])
                mv = wp.tile([P, 2], F32, tag=f"amv{mt}", name=f"amv{mt}")
                nc.vector.bn_aggr(mv[:], st6[:])
                lv = wp.tile([P, 1], F32, tag=f"alv{mt}", name=f"alv{mt}")
                nc.scalar.activation(out=lv[:], in_=mv[:, 1:2], func=AF.Ln,
                                     bias=eps_col[:])
                rs = wp.tile([P, 1], F32, tag=f"ars{mt}", name=f"ars{mt}")
                nc.scalar.activation(out=rs[:], in_=lv[:], func=AF.Exp,
                                     bias=zero_col[:], scale=-0.5)
                al = wp.tile([P, D], F32, tag=f"Aln{mt}", name=f"Aln{mt}")
                nc.vector.tensor_scalar(al[:], pa[:], mv[:, 0:1], rs[:],
                                        op0=ALU.subtract, op1=ALU.mult)
                Aln_s.append(al)

            # ---- Aln^T via PE transpose ([d, t]) ----------------------------
            AlnT_s = [wp.tile([P, T], FDT, tag=f"AlnT{k}", name=f"AlnT{k}") for k in range(2)]
            for mt in range(2):
                for dt_ in range(2):
                    ptr = ps256.tile([P, P], F32, tag="ps", name="atr")
                    nc.tensor.transpose(ptr[:], Aln_s[mt][:, dt_ * P:(dt_ + 1) * P],
                                        ident_s[:])
                    nc.vector.tensor_copy(
                        AlnT_s[dt_][:, mt * P:(mt + 1) * P], ptr[:])

            # ---- y^T = relu(Dy ln(A)^T) o X^T -------------------------------
            yT_s = []
            for m in range(8):
                py = ps256.tile([P, T], F32, tag="ps", name="py")
                for k in range(2):
                    _mm(nc, py[:], DyT_s[k][:, m * P:(m + 1) * P], AlnT_s[k][:],
                        start=(k == 0), stop=(k == 1), fast=use_f32r)
                yt = wp.tile([P, T], FDT, tag=f"yT{m}", name=f"yT{m}")
                nc.vector.scalar_tensor_tensor(
                    out=yt[:], in0=py[:], scalar=0.0, in1=XT_s[m][:].bitcast(F32),
                    op0=ALU.max, op1=ALU.mult)
                yT_s.append(yt)

            # ---- v = y E^T ([t, d]) + layernorm + store ---------------------
            for mt in range(2):
                pv = ps256.tile([P, D], F32, tag="ps", name="pv")
                for k in range(8):
                    _mm(nc, pv[:], yT_s[k][:, mt * P:(mt + 1) * P], ET_s[k][:],
                        start=(k == 0), stop=(k == 7), fast=use_f32r)
                st6 = wp.tile([P, 6], F32, tag=f"ost{mt}", name=f"ost{mt}")
                nc.vector.bn_stats(st6[:], pv[:])
                mv = wp.tile([P, 2], F32, tag=f"omv{mt}", name=f"omv{mt}")
                nc.vector.bn_aggr(mv[:], st6[:])
                lv = wp.tile([P, 1], F32, tag=f"olv{mt}", name=f"olv{mt}")
                nc.scalar.activation(out=lv[:], in_=mv[:, 1:2], func=AF.Ln,
                                     bias=eps_col[:])
                rs = wp.tile([P, 1], F32, tag=f"ors{mt}", name=f"ors{mt}")
                nc.scalar.activation(out=rs[:], in_=lv[:], func=AF.Exp,
                                     bias=zero_col[:], scale=-0.5)
                ov = wp.tile([P, D], F32, tag=f"ov{mt}", name=f"ov{mt}")
                nc.vector.tensor_scalar(ov[:], pv[:], mv[:, 0:1], rs[:],
                                        op0=ALU.subtract, op1=ALU.mult)
                nc.sync.dma_start(d_out[mt * P:(mt + 1) * P, :], ov[:])

    nc.finalize()
    return nc


_NC_CACHE = {}


def _get_nc(use_f32r=USE_F32R):
    if use_f32r not in _NC_CACHE:
        _NC_CACHE[use_f32r] = _build_nc(use_f32r)
    return _NC_CACHE[use_f32r]


def _host_consts():
    ii = np.arange(T, dtype=np.float64)
    ln097 = np.log(np.float64(DECAY))
    maskCT = np.where(ii[:, None] <= ii[None, :], 0.0, -1e30).astype(np.float32)
    DupT = np.where(
        ii[:, None] < ii[None, :],
        np.float64(DECAY) ** (ii[None, :] - 1 - ii[:, None]),
        0.0,
    ).astype(np.float32)
    mask2 = np.ascontiguousarray(np.concatenate([maskCT, DupT], axis=1))
    tid = np.ascontiguousarray(np.concatenate(
        [np.triu(np.ones((P, P), np.float32), k=1), np.eye(P, dtype=np.float32)],
        axis=1))
    sc = np.zeros((T, 4), np.float32)
    sc[:, 0] = DECAY
    sc[0, 0] = 0.0
    sc[:, 1] = (ii * ln097).astype(np.float32)
    sc[:, 2] = (-ii * ln097).astype(np.float32)
    return sc, tid, mask2


def make_in_maps(embeddings, E, Dx, Dy):
    emb = np.ascontiguousarray(np.asarray(embeddings, dtype=np.float32))
    E = np.asarray(E, dtype=np.float32)
    Dx = np.asarray(Dx, dtype=np.float32)
    Dy = np.asarray(Dy, dtype=np.float32)
    sc, tid, mask2 = _host_consts()
    shared = {
        "sc": sc, "tid": tid, "mask2": mask2,
        "DxT": np.ascontiguousarray(Dx.T),
        "DyT": np.ascontiguousarray(Dy.T),
        "ET": np.ascontiguousarray(E.T),
    }
    in_maps = []
    for b in range(B):
        m = dict(shared)
        m["em2"] = np.ascontiguousarray(
            np.concatenate([emb[b], emb[b].T], axis=1))
        in_maps.append(m)
    return in_maps


def kernel(embeddings, E, Dx, Dy, _use_f32r=USE_F32R):
    in_maps = make_in_maps(embeddings, E, Dx, Dy)
    nc = _get_nc(_use_f32r)
    res = run_bass_kernel_spmd(nc, in_maps, core_ids=list(range(B)))
    return np.stack([r["out"] for r in res.results], axis=0)


# revision 20
# speedup vs baseline: 1.2323x; 1.0915x over previous
"""Trainium2 Bass kernel for the BDH recurrent block (B=8, T=256, d=256, n=1024).

Key reformulation: the scan input v_prev is the *embedding* at each step (the
output v_star is never fed back), so the only recurrences are

  x_t = l1norm(0.97 * x_{t-1} + relu(emb_t @ Dx.T))          (elementwise, n)
  rho_t = 0.97 * rho_{t-1} + ln(emb_t) (x) x_t               (rank-1, d*n)

Both have closed forms:
  x_t  = sum_s C[t,s] * U_s           with U = relu(emb @ Dx.T)  and
         C[t,s] = 0.97^{t-s} / prod_{r=s..t} b_r,  b_r = sum(U_r) + 0.97*[r>0]
         (b_0 = sum(U_0)), computed in log space via a cumulative sum.
  a*_t = rho_{t-1} x_t = sum_{s<t} 0.97^{t-1-s} (x_s . x_t) ln(emb_s)
       = ((X X^T) o D) @ ln(emb)     -- decay-masked attention.

So the whole T-step scan becomes a handful of dense matmuls, one sample per
NeuronCore (data-parallel over B=8 across 8 cores, weights replicated).
"""

import numpy as np

import concourse.bass as bass
import concourse.tile as tile
from concourse import bacc, mybir
from concourse.bass_utils import run_bass_kernel_spmd
from concourse.hw_specs import get_activation_tables

B, T, D, N = 8, 256, 256, 1024
P = 128  # partitions
LN_EPS = 1e-5
DECAY = 0.97
F32 = mybir.dt.float32
F32R = mybir.dt.float32r
AF = mybir.ActivationFunctionType
ALU = mybir.AluOpType

# fp32r runs the PE at 4x the fp32 rate (1 cycle/row at N>=256); inputs are
# fp32 in SBUF, bitcast at the matmul. Used only for the big matmuls.
USE_F32R = True


def _mm(nc, out, lhsT, rhs, start, stop, fast):
    nc.tensor.matmul(out, lhsT, rhs, start=start, stop=stop)


def _build_nc(use_f32r=USE_F32R):
    nc = bacc.Bacc()
    FDT = F32R if use_f32r else F32

    # packed inputs (few large DMAs, ordered by when the pipeline needs them)
    d_tid = nc.dram_tensor("tid", [P, 2 * P], F32, kind="ExternalInput")  # triu|ident
    d_em2 = nc.dram_tensor("em2", [T, 4 + 2 * D], FDT, kind="ExternalInput")  # sc|emb|embT
    d_DxT = nc.dram_tensor("DxT", [D, N], FDT, kind="ExternalInput")
    d_mask2 = nc.dram_tensor("mask2", [T, 2 * T], F32, kind="ExternalInput")  # maskCT|DupT
    d_DyT = nc.dram_tensor("DyT", [D, N], FDT, kind="ExternalInput")
    d_ET = nc.dram_tensor("ET", [N, D], FDT, kind="ExternalInput")
    d_out = nc.dram_tensor("out", [T, D], F32, kind="ExternalOutput")

    # Preload the one ACT table set containing every function we use
    # (relu/ln/exp/copy) so the compiler never swaps tables mid-kernel
    # (each swap costs ~2.7us on the Scalar engine).
    act_sets = list(get_activation_tables(nc.m.arch))
    combined_set_id = act_sets.index("natural_log_exp_and_others")

    with tile.TileContext(nc) as tc:
        with (
            tc.tile_pool(name="consts", bufs=1) as cp,
            tc.tile_pool(name="work", bufs=1) as wp,
            tc.tile_pool(name="ps512", bufs=2, space="PSUM") as ps512,
            tc.tile_pool(name="ps256", bufs=4, space="PSUM") as ps256,
            tc.tile_pool(name="ps_small", bufs=2, space="PSUM") as pss,
        ):
            # ---- load inputs (issue order == need order) --------------------
            def load2(dram, f, tag, dt_=F32):  # (2P, f) dram -> two [P, f] tiles
                ts = []
                for k in range(2):
                    t = cp.tile([P, f], dt_, tag=f"{tag}{k}", name=f"{tag}{k}")
                    nc.sync.dma_start(t[:], dram[k * P:(k + 1) * P, :])
                    ts.append(t)
                return ts

            def load_split(dram, f, tag, dt_=F32):
                # tile k=0 issued on SP, k=1 on ACT (both are HWDGE-capable)
                ts = []
                for k, eng in ((0, nc.sync), (1, nc.scalar)):
                    t = cp.tile([P, f], dt_, tag=f"{tag}{k}", name=f"{tag}{k}")
                    eng.dma_start(t[:], dram[k * P:(k + 1) * P, :])
                    ts.append(t)
                return ts

            em2_s = load_split(d_em2, 4 + 2 * D, "em2", FDT)
            c097_s = [t[:, 0:1].bitcast(F32) for t in em2_s]
            iotaP_s = [t[:, 1:2].bitcast(F32) for t in em2_s]
            iotaQ_s = [t[:, 2:3].bitcast(F32) for t in em2_s]
            emb_s = [t[:, 4:4 + D].bitcast(F32) for t in em2_s]
            embT_s = [t[:, 4 + D:4 + 2 * D] for t in em2_s]
            DxT_s = load_split(d_DxT, N, "DxT", FDT)
            tid_s = cp.tile([P, 2 * P], F32, tag="tid", name="tid")
            nc.scalar.dma_start(tid_s[:], d_tid[:, :])
            triu_s = tid_s[:, 0:P]
            ident_s = tid_s[:, P:2 * P]
            mask2_s = load_split(d_mask2, 2 * T, "mask2")
            maskCT_s = [t[:, 0:T] for t in mask2_s]
            DupT_s = [t[:, T:2 * T] for t in mask2_s]
            DyT_s = load_split(d_DyT, N, "DyT", FDT)
            et_big = cp.tile([P, 8, D], FDT, tag="et_big", name="et_big")
            nc.sync.dma_start(
                et_big[:], d_ET.rearrange("(k p) d -> p k d", p=P))
            ET_s = [et_big[:, k, :] for k in range(8)]
            nc.scalar.add_instruction(mybir.InstLoadActFuncSet(
                name=nc.get_next_instruction_name(),
                act_func_set_id=combined_set_id, ins=[], outs=[]))
            ones_blk = cp.tile([P, P], F32, tag="ones_blk", name="ones_blk")
            nc.vector.memset(ones_blk[:], 1.0)


            ones_row = cp.tile([1, P], F32, tag="ones_row", name="ones_row")
            nc.vector.memset(ones_row[:], 1.0)
            zero_col = cp.tile([P, 1], F32, tag="zero_col", name="zero_col")
            nc.vector.memset(zero_col[:], 0.0)
            eps_col = cp.tile([P, 1], F32, tag="eps_col", name="eps_col")
            nc.vector.memset(eps_col[:], LN_EPS)

            # ---- U = relu(emb @ Dx.T), row sums a ---------------------------
            U_s = [wp.tile([P, N], FDT, tag=f"U{m}", name=f"U{m}") for m in range(2)]
            a_s = [wp.tile([P, 1], F32, tag=f"a{m}", name=f"a{m}") for m in range(2)]
            apart = [[wp.tile([P, 1], F32, tag=f"ap{m}{c}", name=f"ap{m}{c}") for c in range(2)]
                     for m in range(2)]
            for mt in range(2):
                for ch in range(2):
                    pu = ps512.tile([P, 512], F32, tag="pu", name="pu")
                    for k in range(2):
                        _mm(nc, pu[:], embT_s[k][:, mt * P:(mt + 1) * P],
                            DxT_s[k][:, ch * 512:(ch + 1) * 512],
                            start=(k == 0), stop=(k == 1), fast=use_f32r)
                    nc.scalar.activation(
                        out=U_s[mt][:, ch * 512:(ch + 1) * 512], in_=pu[:],
                        func=AF.Relu, bias=zero_col[:], accum_out=apart[mt][ch][:])
                nc.vector.tensor_add(a_s[mt][:], apart[mt][0][:], apart[mt][1][:])

            # ---- scalar chain: b, log b, cumsum, p, q ------------------------
            logb_s = []
            q_s = []
            p_s = []
            for mt in range(2):
                bvec = wp.tile([P, 1], F32, tag=f"b{mt}", name=f"b{mt}")
                nc.vector.tensor_add(bvec[:], a_s[mt][:], c097_s[mt][:])
                lb = wp.tile([P, 1], F32, tag=f"lb{mt}", name=f"lb{mt}")
                nc.scalar.activation(out=lb[:], in_=bvec[:], func=AF.Ln, bias=zero_col[:])
                logb_s.append(lb)
            for mt in range(2):
                pl = pss.tile([P, 1], F32, tag="pss", name="plam")
                if mt == 0:
                    nc.tensor.matmul(pl[:], triu_s[:], logb_s[0][:],
                                     start=True, stop=True)
                else:
                    nc.tensor.matmul(pl[:], ones_blk[:], logb_s[0][:],
                                     start=True, stop=False)
                    nc.tensor.matmul(pl[:], triu_s[:], logb_s[1][:],
                                     start=False, stop=True)
                # q = lamS + iotaQ ; p = (iotaP - lamS) - logb   (lamS in PSUM)
                qv = wp.tile([P, 1], F32, tag=f"q{mt}", name=f"q{mt}")
                nc.vector.tensor_add(qv[:], pl[:], iotaQ_s[mt][:])
                q_s.append(qv)
                pv = wp.tile([P, 1], F32, tag=f"p{mt}", name=f"p{mt}")
                nc.vector.scalar_tensor_tensor(
                    out=pv[:], in0=iotaP_s[mt][:], scalar=pl[:],
                    in1=logb_s[mt][:], op0=ALU.subtract, op1=ALU.subtract)
                p_s.append(pv)

            # ---- p as a row vector (PE transpose) ---------------------------
            p_row = wp.tile([1, T], F32, tag="p_row", name="p_row")
            for mt in range(2):
                pt = pss.tile([1, P], F32, tag="pss", name="ptr")
                nc.tensor.transpose(pt[:], p_s[mt][:], ident_s[:])
                nc.vector.tensor_copy(p_row[:, mt * P:(mt + 1) * P], pt[:])

            # ---- CT[s,t] = exp(q_s + p_t + mask) ----------------------------
            CT_s = []
            for st in range(2):
                pb = ps256.tile([P, T], F32, tag="ps", name="pb")
                nc.tensor.matmul(pb[:], ones_row[:], p_row[:],
                                 start=True, stop=True)
                tmp = wp.tile([P, T], F32, tag=f"ctmp{st}", name=f"ctmp{st}")
                nc.vector.tensor_add(tmp[:], pb[:], maskCT_s[st][:])
                ct = wp.tile([P, T], FDT, tag=f"CT{st}", name=f"CT{st}")
                nc.scalar.activation(out=ct[:], in_=tmp[:], func=AF.Exp,
                                     bias=q_s[st][:], scale=1.0)
                CT_s.append(ct)

            # ---- X^T = U^T C^T  (n on partitions, T free) -------------------
            XT_s = []
            for m in range(8):
                px = ps256.tile([P, T], F32, tag="ps", name="px")
                for k in range(2):
                    _mm(nc, px[:], U_s[k][:, m * P:(m + 1) * P], CT_s[k][:],
                        start=(k == 0), stop=(k == 1), fast=use_f32r)
                xt = wp.tile([P, T], FDT, tag=f"XT{m}", name=f"XT{m}")
                if m % 2 == 0:
                    nc.vector.tensor_copy(xt[:], px[:])
                else:
                    nc.scalar.copy(xt[:], px[:])
                XT_s.append(xt)

            # ---- W = ln(emb rows) -------------------------------------------
            W_s = []
            for mt in range(2):
                st6 = wp.tile([P, 6], F32, tag=f"wst{mt}", name=f"wst{mt}")
                nc.vector.bn_stats(st6[:], emb_s[mt][:])
                mv = wp.tile([P, 2], F32, tag=f"wmv{mt}", name=f"wmv{mt}")
                nc.vector.bn_aggr(mv[:], st6[:])
                lv = wp.tile([P, 1], F32, tag=f"wlv{mt}", name=f"wlv{mt}")
                nc.scalar.activation(out=lv[:], in_=mv[:, 1:2], func=AF.Ln,
                                     bias=eps_col[:])
                rs = wp.tile([P, 1], F32, tag=f"wrs{mt}", name=f"wrs{mt}")
                nc.scalar.activation(out=rs[:], in_=lv[:], func=AF.Exp,
                                     bias=zero_col[:], scale=-0.5)
                w = wp.tile([P, D], FDT, tag=f"W{mt}", name=f"W{mt}")
                nc.vector.tensor_scalar(w[:], emb_s[mt][:], mv[:, 0:1], rs[:],
                                        op0=ALU.subtract, op1=ALU.mult)
                W_s.append(w)

            # ---- G = X X^T ; GD = G o Dup -----------------------------------
            GD_s = []
            for st in range(2):
                pg = ps256.tile([P, T], F32, tag="ps", name="pg")
                for k in range(8):
                    _mm(nc, pg[:], XT_s[k][:, st * P:(st + 1) * P], XT_s[k][:],
                        start=(k == 0), stop=(k == 7), fast=use_f32r)
                gd = wp.tile([P, T], FDT, tag=f"GD{st}", name=f"GD{st}")
                nc.vector.tensor_mul(gd[:], pg[:], DupT_s[st][:])
                GD_s.append(gd)

            # ---- A = (G o D) @ W  ([t, d]) + layernorm ----------------------
            Aln_s = []
            for mt in range(2):
                pa = ps256.tile([P, D], F32, tag="ps", name="pa")
                ks = [0] if mt == 0 else [0, 1]
                for k in ks:
                    _mm(nc, pa[:], GD_s[k][:, mt * P:(mt + 1) * P], W_s[k][:],
                        start=(k == ks[0]), stop=(k == ks[-1]), fast=use_f32r)
                st6 = wp.tile([P, 6], F32, tag=f"ast{mt}", name=f"ast{mt}")
                nc.vector.bn_stats(st6[:], pa[:])
                mv = wp.tile([P, 2], F32, tag=f"amv{mt}", name=f"amv{mt}")
                nc.vector.bn_aggr(mv[:], st6[:])
                lv = wp.tile([P, 1], F32, tag=f"alv{mt}", name=f"alv{mt}")
                nc.scalar.activation(out=lv[:], in_=mv[:, 1:2], func=AF.Ln,
                                     bias=eps_col[:])
                rs = wp.tile([P, 1], F32, tag=f"ars{mt}", name=f"ars{mt}")
                nc.scalar.activation(out=rs[:], in_=lv[:], func=AF.Exp,
                                     bias=zero_col[:], scale=-0.5)
                al = wp.tile([P, D], F32, tag=f"Aln{mt}", name=f"Aln{mt}")
                nc.vector.tensor_scalar(al[:], pa[:], mv[:, 0:1], rs[:],
                                        op0=ALU.subtract, op1=ALU.mult)
                Aln_s.append(al)

            # ---- Aln^T via PE transpose ([d, t]) ----------------------------
            AlnT_s = [wp.tile([P, T], FDT, tag=f"AlnT{k}", name=f"AlnT{k}") for k in range(2)]
            for mt in range(2):
                for dt_ in range(2):
                    ptr = ps256.tile([P, P], F32, tag="ps", name="atr")
                    nc.tensor.transpose(ptr[:], Aln_s[mt][:, dt_ * P:(dt_ + 1) * P],
                                        ident_s[:])
                    nc.vector.tensor_copy(
                        AlnT_s[dt_][:, mt * P:(mt + 1) * P], ptr[:])

            # ---- y^T = relu(Dy ln(A)^T) o X^T -------------------------------
            yT_s = []
            for m in range(8):
                py = ps256.tile([P, T], F32, tag="ps", name="py")
                for k in range(2):
                    _mm(nc, py[:], DyT_s[k][:, m * P:(m + 1) * P], AlnT_s[k][:],
                        start=(k == 0), stop=(k == 1), fast=use_f32r)
                yt = wp.tile([P, T], FDT, tag=f"yT{m}", name=f"yT{m}")
                nc.vector.scalar_tensor_tensor(
                    out=yt[:], in0=py[:], scalar=0.0, in1=XT_s[m][:].bitcast(F32),
                    op0=ALU.max, op1=ALU.mult)
                yT_s.append(yt)

            # ---- v = y E^T ([t, d]) + layernorm + store ---------------------
            for mt in range(2):
                pv = ps256.tile([P, D], F32, tag="ps", name="pv")
                for k in range(8):
                    _mm(nc, pv[:], yT_s[k][:, mt * P:(mt + 1) * P], ET_s[k][:],
                        start=(k == 0), stop=(k == 7), fast=use_f32r)
                st6 = wp.tile([P, 6], F32, tag=f"ost{mt}", name=f"ost{mt}")
                nc.vector.bn_stats(st6[:], pv[:])
                mv = wp.tile([P, 2], F32, tag=f"omv{mt}", name=f"omv{mt}")
                nc.vector.bn_aggr(mv[:], st6[:])
                lv = wp.tile([P, 1], F32, tag=f"olv{mt}", name=f"olv{mt}")
                nc.scalar.activation(out=lv[:], in_=mv[:, 1:2], func=AF.Ln,
                                     bias=eps_col[:])
                rs = wp.tile([P, 1], F32, tag=f"ors{mt}", name=f"ors{mt}")
                nc.scalar.activation(out=rs[:], in_=lv[:], func=AF.Exp,
                                     bias=zero_col[:], scale=-0.5)
                ov = wp.tile([P, D], F32, tag=f"ov{mt}", name=f"ov{mt}")
                nc.vector.tensor_scalar(ov[:], pv[:], mv[:, 0:1], rs[:],
                                        op0=ALU.subtract, op1=ALU.mult)
                nc.sync.dma_start(d_out[mt * P:(mt + 1) * P, :], ov[:])

    nc.finalize()
    return nc


_NC_CACHE = {}


def _get_nc(use_f32r=USE_F32R):
    if use_f32r not in _NC_CACHE:
        _NC_CACHE[use_f32r] = _build_nc(use_f32r)
    return _NC_CACHE[use_f32r]


def _host_consts():
    ii = np.arange(T, dtype=np.float64)
    ln097 = np.log(np.float64(DECAY))
    maskCT = np.where(ii[:, None] <= ii[None, :], 0.0, -1e30).astype(np.float32)
    DupT = np.where(
        ii[:, None] < ii[None, :],
        np.float64(DECAY) ** (ii[None, :] - 1 - ii[:, None]),
        0.0,
    ).astype(np.float32)
    mask2 = np.ascontiguousarray(np.concatenate([maskCT, DupT], axis=1))
    tid = np.ascontiguousarray(np.concatenate(
        [np.triu(np.ones((P, P), np.float32), k=1), np.eye(P, dtype=np.float32)],
        axis=1))
    sc = np.zeros((T, 4), np.float32)
    sc[:, 0] = DECAY
    sc[0, 0] = 0.0
    sc[:, 1] = (ii * ln097).astype(np.float32)
    sc[:, 2] = (-ii * ln097).astype(np.float32)
    return sc, tid, mask2


def make_in_maps(embeddings, E, Dx, Dy):
    emb = np.ascontiguousarray(np.asarray(embeddings, dtype=np.float32))
    E = np.asarray(E, dtype=np.float32)
    Dx = np.asarray(Dx, dtype=np.float32)
    Dy = np.asarray(Dy, dtype=np.float32)
    sc, tid, mask2 = _host_consts()
    shared = {
        "tid": tid, "mask2": mask2,
        "DxT": np.ascontiguousarray(Dx.T),
        "DyT": np.ascontiguousarray(Dy.T),
        "ET": np.ascontiguousarray(E.T),
    }
    in_maps = []
    for b in range(B):
        m = dict(shared)
        m["em2"] = np.ascontiguousarray(
            np.concatenate([sc, emb[b], emb[b].T], axis=1))
        in_maps.append(m)
    return in_maps


def kernel(embeddings, E, Dx, Dy, _use_f32r=USE_F32R):
    in_maps = make_in_maps(embeddings, E, Dx, Dy)
    nc = _get_nc(_use_f32r)
    res = run_bass_kernel_spmd(nc, in_maps, core_ids=list(range(B)))
    return np.stack([r["out"] for r in res.results], axis=0)
